# revision 1
# baseline (speedup 1.0000x reference)
"""Trainium2 Bass kernel for nn_CPCircuitLayer (embedding_lookup).

Math: A = X @ W_seq^T  [S,R];  Bm = X^T @ W_hid^T  [H,R]
      out[b, n] = dot(A[b, idx_s[n]], Bm[b, idx_h[n]]),  out -> [B, S, H]

Sharding (8 cores, no collectives): core c handles batch b = c//4 and the
quarter q = c%4 of the N = S*H index list (J = N/4 indices). Both factor
tables are computed redundantly per batch group from the full X[b].

Per-core device pipeline:
  1. Load X[b] (bf16) + transposed copy via HWDGE transpose-DMA.
  2. PE matmuls (bf16 in, f32 psum): A^T and Bm^T [32, 1024].
  3. Repack to per-lane split-R tables: partition p holds columns
     2*(p%16), 2*(p%16)+1 of the factor interleaved ([128, 1024, 2] f32),
     via a DRAM bounce + 8x partition-group broadcast load.
  4. ap_gather (GPSIMD FIFO): each 16-partition group streams its own
     indices; one instruction gathers NIdx rows x 8 groups.
  5. DVE mul + pair-sum, then PE block-indicator matmul reduces the 16
     lanes x 2 of each group -> psum [8, n] -> out.
"""

import numpy as np
import ml_dtypes
from contextlib import ExitStack

import concourse.bass as bass
import concourse.mybir as mybir
import concourse.tile as tile
from concourse import bacc

B, S, H, R = 2, 1024, 1024, 32
N = S * H
NCORES = 8
J = N // 4            # outputs per core (one batch, quarter of N) = 262144
JG = J // 8           # outputs per 16-partition group = 32768
NIdx = 2048           # indices per group per ap_gather instruction
RNDS = JG // NIdx     # 16 gather rounds per table
GRP_D = 2             # table f32 per lane (R = 16 lanes * 2)
SKIP_GATHER = False   # timing experiment: drop ap_gather instructions

F32 = mybir.dt.float32
BF16 = mybir.dt.bfloat16
I16 = mybir.dt.int16


def _build(reps: int = 1):
    nc = bacc.Bacc()
    x = nc.declare_dram_parameter("x", [S, H], BF16, False)
    wseq_t = nc.declare_dram_parameter("wseq_t", [H, R], BF16, False)
    whid_t = nc.declare_dram_parameter("whid_t", [S, R], BF16, False)
    # per-group index streams, wrapped: group g's jj-th index lives at
    # [16*g + jj%16, jj//16]
    idx_s = nc.declare_dram_parameter("idx_s", [128, 2 * JG // 16], I16, False)
    idx_h = nc.declare_dram_parameter("idx_h", [128, 2 * JG // 16], I16, False)
    ind_in = nc.declare_dram_parameter("ind", [128, 8], F32, False)
    out = nc.declare_dram_parameter("out", [8, JG], F32, True)
    ta_dram = nc.dram_tensor("ta", [R, S], F32)   # A^T bounce
    tb_dram = nc.dram_tensor("tb", [R, H], F32)   # Bm^T bounce

    with tile.TileContext(nc) as tc, ExitStack() as ctx:
        base = ctx.enter_context(tc.tile_pool(name="base", bufs=1))
        psum = ctx.enter_context(tc.tile_pool(name="psum", bufs=2, space="PSUM"))
        rpsum = ctx.enter_context(tc.tile_pool(name="rpsum", bufs=1, space="PSUM"))
        stage = ctx.enter_context(tc.tile_pool(name="stage", bufs=2))
        gap = ctx.enter_context(tc.tile_pool(name="gap", bufs=2))
        gbp = ctx.enter_context(tc.tile_pool(name="gbp", bufs=2))
        prodp = ctx.enter_context(tc.tile_pool(name="prodp", bufs=2))
        otp = ctx.enter_context(tc.tile_pool(name="otp", bufs=1))

        # --- loads -------------------------------------------------------
        x_sb = base.tile([128, 8, H], BF16)       # X[s,h]: p=s%128, k=s//128
        xt_sb = base.tile([128, 8, S], BF16)      # X^T[h,s]: p=h%128, k=h//128
        ws_sb = base.tile([128, 8, R], BF16)      # W_seq^T rows (h-major)
        wh_sb = base.tile([128, 8, R], BF16)      # W_hid^T rows (s-major)
        isb_s = base.tile([128, 2 * JG // 16], I16)
        isb_h = base.tile([128, 2 * JG // 16], I16)
        ind_sb = base.tile([128, 8], F32)         # block indicator for reduce
        ta_sb = base.tile([128, 2 * S], F32)
        tb_sb = base.tile([128, 2 * H], F32)

        nc.sync.dma_start(
            out=x_sb[:],
            in_=bass.AP(tensor=x[:].tensor, offset=0,
                        ap=[[H, 128], [128 * H, 8], [1, H]]),
        )
        for k in range(8):
            nc.sync.dma_start_transpose(
                out=xt_sb[:, k, :], in_=x[:, 128 * k:128 * (k + 1)]
            )
        nc.sync.dma_start(
            out=ws_sb[:],
            in_=bass.AP(tensor=wseq_t[:].tensor, offset=0,
                        ap=[[R, 128], [128 * R, 8], [1, R]]),
        )
        nc.sync.dma_start(
            out=wh_sb[:],
            in_=bass.AP(tensor=whid_t[:].tensor, offset=0,
                        ap=[[R, 128], [128 * R, 8], [1, R]]),
        )
        nc.sync.dma_start(out=isb_s[:], in_=idx_s[:])
        nc.sync.dma_start(out=isb_h[:], in_=idx_h[:])

        nc.sync.dma_start(out=ind_sb[:], in_=ind_in[:])

        for _ in range(reps):
            _body(nc, psum, rpsum, stage, gap, gbp, prodp, otp,
                  x_sb, xt_sb, ws_sb, wh_sb, isb_s, isb_h, ind_sb,
                  ta_sb, tb_sb, ta_dram, tb_dram, out)
    nc.compile()
    return nc


def _body(nc, psum, rpsum, stage, gap, gbp, prodp, otp,
          x_sb, xt_sb, ws_sb, wh_sb, isb_s, isb_h, ind_sb,
          ta_sb, tb_sb, ta_dram, tb_dram, out):
    # --- factor transposes on PE: F^T [32, 1024] ------------------------
    # A^T[r, s] = sum_h Wseq^T[h, r] X^T[h, s]; Bm^T[r, h] = sum_s ...
    for (tdram, lhs_w, rhs_x) in ((ta_dram, ws_sb, xt_sb),
                                  (tb_dram, wh_sb, x_sb)):
        ft = stage.tile([R, S], F32, tag="ft")
        for nh in range(2):
            pt = psum.tile([R, S // 2], F32, tag="pt")
            for k in range(8):
                nc.tensor.matmul(
                    out=pt[:],
                    lhsT=lhs_w[:, k, :],
                    rhs=rhs_x[:, k, nh * 512:(nh + 1) * 512],
                    start=(k == 0), stop=(k == 7),
                )
            nc.vector.tensor_copy(out=ft[:, nh * 512:(nh + 1) * 512], in_=pt[:])
        nc.gpsimd.dma_start(out=tdram[:], in_=ft[:])

    # broadcast tables back, lane-split d=2 interleaved: partition p
    # (lane l = p%16) holds tab[p, v, d] = F^T[2l+d, v]
    for (tdram, tsb, V) in ((ta_dram, ta_sb, S), (tb_dram, tb_sb, H)):
        nc.gpsimd.dma_start(
            out=tsb[:],
            in_=bass.AP(tensor=tdram[:].tensor, offset=0,
                        ap=[[0, 8], [2 * V, 16], [1, 2 * V]]),
        )

    # --- gather + reduce ------------------------------------------------
    ot = None
    for rnd in range(RNDS):
        isl = slice(rnd * (2 * NIdx // 16), (rnd + 1) * (2 * NIdx // 16))
        ga = gap.tile([128, NIdx, GRP_D], F32, tag="ga")
        gb = gbp.tile([128, NIdx, GRP_D], F32, tag="gb")
        ga_flat = bass.AP(tensor=ga[:].tensor, offset=ga[:].offset,
                          ap=[list(ga[:].ap[0]), [1, 2 * NIdx], [1, 1]])
        gb_flat = bass.AP(tensor=gb[:].tensor, offset=gb[:].offset,
                          ap=[list(gb[:].ap[0]), [1, 2 * NIdx], [1, 1]])
        if SKIP_GATHER:
            nc.vector.memset(ga[:], 0.0)
            nc.vector.memset(gb[:], 0.0)
        else:
            nc.gpsimd.ap_gather(
                out_ap=ga_flat, in_ap=ta_sb[:], idxs_ap=isb_s[:, isl],
                channels=128, num_elems=2 * S, d=1, num_idxs=2 * NIdx,
            )
            nc.gpsimd.ap_gather(
                out_ap=gb_flat, in_ap=tb_sb[:], idxs_ap=isb_h[:, isl],
                channels=128, num_elems=2 * H, d=1, num_idxs=2 * NIdx,
            )
        prod = prodp.tile([128, NIdx, GRP_D], F32, tag="prod")
        nc.vector.tensor_mul(prod[:], ga[:], gb[:])
        p2 = prodp.tile([128, NIdx], F32, tag="p2")
        nc.vector.tensor_add(p2[:], prod[:, :, 0], prod[:, :, 1])
        # reduce 16 lanes per group via block-indicator matmul; all four
        # 512-col results land in one 4-bank psum tile -> single copy;
        # out-DMA once per two rounds
        if rnd % 2 == 0:
            ot = otp.tile([8, 2 * NIdx], F32, tag="ot")
        rp4 = rpsum.tile([8, NIdx], F32, tag="rp4")
        for t in range(NIdx // 512):
            nc.tensor.matmul(
                out=rp4[:, t * 512:(t + 1) * 512],
                lhsT=ind_sb[:],
                rhs=p2[:, t * 512:(t + 1) * 512],
                start=True, stop=True,
            )
        half = (rnd % 2) * NIdx
        nc.scalar.copy(out=ot[:, half:half + NIdx], in_=rp4[:])
        if rnd % 2 == 1:
            nc.sync.dma_start(
                out=bass.AP(tensor=out[:].tensor, offset=(rnd - 1) * NIdx,
                            ap=[[JG, 8], [1, 2 * NIdx]]),
                in_=ot[:],
            )


_nc_cache_by_reps = {}


def _get_nc(reps: int = 1):
    nc = _nc_cache_by_reps.get(reps)
    if nc is None:
        nc = _nc_cache_by_reps[reps] = _build(reps)
    return nc


class _Runner:
    """Trace/compile the SPMD executable once; reuse across calls."""

    def __init__(self, nc):
        import jax
        from jax.experimental.shard_map import shard_map
        from jax.sharding import Mesh, PartitionSpec
        import concourse.bass2jax as b2j

        b2j.install_neuronx_cc_hook()
        self.nc = nc
        part_name = (nc.partition_id_tensor.name
                     if nc.partition_id_tensor else None)
        in_names, out_names, out_avals = [], [], []
        zero_outs = []
        for alloc in nc.m.functions[0].allocations:
            if not isinstance(alloc, mybir.MemoryLocationSet):
                continue
            name = alloc.memorylocations[0].name
            if alloc.kind == "ExternalInput":
                if name != part_name:
                    in_names.append(name)
            elif alloc.kind == "ExternalOutput":
                out_names.append(name)
                shape = tuple(alloc.tensor_shape)
                dtype = mybir.dt.np(alloc.dtype)
                out_avals.append(jax.core.ShapedArray(shape, dtype))
                zero_outs.append(np.zeros(shape, dtype))
        self.in_names = list(in_names)
        self.out_names = out_names
        self.zero_outs = zero_outs
        n_params = len(in_names)
        n_outs = len(out_names)
        all_in_names = in_names + out_names
        if part_name is not None:
            all_in_names = all_in_names + [part_name]
        donate = tuple(range(n_params, n_params + n_outs))

        def _body_fn(*args):
            operands = list(args)
            if part_name is not None:
                operands.append(b2j.partition_id_tensor())
            outs = b2j._bass_exec_p.bind(
                *operands,
                out_avals=tuple(out_avals),
                in_names=tuple(all_in_names),
                out_names=tuple(out_names),
                lowering_input_output_aliases=(),
                sim_require_finite=True,
                sim_require_nnan=True,
                nc=nc,
            )
            return tuple(outs)

        devices = jax.devices()[:NCORES]
        mesh = Mesh(np.asarray(devices), ("core",))
        self.fn = jax.jit(
            shard_map(
                _body_fn, mesh=mesh,
                in_specs=(PartitionSpec("core"),) * (n_params + n_outs),
                out_specs=(PartitionSpec("core"),) * n_outs,
                check_rep=False,
            ),
            donate_argnums=donate,
            keep_unused=True,
        )

    def __call__(self, in_maps):
        concat_in = [
            np.concatenate([np.asarray(m[name]) for m in in_maps], axis=0)
            for name in self.in_names
        ]
        concat_zeros = [
            np.zeros((NCORES * z.shape[0], *z.shape[1:]), z.dtype)
            for z in self.zero_outs
        ]
        out_arrs = self.fn(*concat_in, *concat_zeros)
        return [
            {
                name: np.asarray(out_arrs[i]).reshape(NCORES, -1)[c]
                for i, name in enumerate(self.out_names)
            }
            for c in range(NCORES)
        ]


_runner_cache = {}


def _get_runner(reps: int = 1):
    r = _runner_cache.get(reps)
    if r is None:
        r = _runner_cache[reps] = _Runner(_get_nc(reps))
    return r


def _wrap_idx(v: np.ndarray) -> np.ndarray:
    """[J] -> [128, 2*JG/16] int16: group g = j // JG streams the pairs
    (v, v+1024) for its outputs, wrapped at [16*g + t%16, t//16]."""
    v = v.astype(np.int16)
    v2 = np.empty(2 * J, np.int16)
    v2[0::2] = v
    v2[1::2] = v + 1024
    w = v2.reshape(8, 2 * JG // 16, 16)   # [g, col, p16]
    w = w.transpose(0, 2, 1).reshape(128, 2 * JG // 16)
    return np.ascontiguousarray(w)


def prepare_in_maps(hidden_states, W_seq, W_hid, all_indices):
    x_bf = [np.ascontiguousarray(hidden_states[b].astype(ml_dtypes.bfloat16))
            for b in range(B)]
    ws_t = np.ascontiguousarray(W_seq.T.astype(ml_dtypes.bfloat16))
    wh_t = np.ascontiguousarray(W_hid.T.astype(ml_dtypes.bfloat16))
    idx_pairs = []
    for q in range(4):
        seg = all_indices[q * J:(q + 1) * J]
        idx_pairs.append((_wrap_idx(seg[:, 0]), _wrap_idx(seg[:, 1])))
    in_maps = []
    for c in range(NCORES):
        b, q = c // 4, c % 4
        ind = np.zeros((128, 8), np.float32)
        for g in range(8):
            ind[16 * g:16 * (g + 1), g] = 1.0
        in_maps.append({
            "x": x_bf[b],
            "wseq_t": ws_t,
            "whid_t": wh_t,
            "idx_s": idx_pairs[q][0],
            "idx_h": idx_pairs[q][1],
            "ind": ind,
        })
    return in_maps


def kernel(hidden_states, W_seq, W_hid, all_indices):
    hidden_states = np.asarray(hidden_states)
    W_seq = np.asarray(W_seq)
    W_hid = np.asarray(W_hid)
    all_indices = np.asarray(all_indices)

    runner = _get_runner()
    in_maps = prepare_in_maps(hidden_states, W_seq, W_hid, all_indices)
    results = runner(in_maps)

    out = np.empty((B, N), dtype=np.float32)
    for c in range(NCORES):
        b, q = c // 4, c % 4
        o = results[c]["out"].reshape(8, JG)
        # out[g, jj] holds output j = g*JG + jj of this core's quarter
        out[b, q * J:(q + 1) * J] = o.reshape(J)
    return out.reshape(B, S, H)



# revision 2
# speedup vs baseline: 41.8783x; 41.8783x over previous
"""Trainium2 Bass kernel for nn_CPCircuitLayer (embedding_lookup).

Math: A_b = X_b @ W_seq^T [S,R]; Bm_b = X_b^T @ W_hid^T [H,R]
      out[b, n] = dot(A_b[idx_s[n]], Bm_b[idx_h[n]]),  out -> [B, S, H]

Key reformulation: out[b, n] = G_b[idx_s[n], idx_h[n]] where
G_b = A_b @ Bm_b^T is a [S, H] = [1024, 1024] f32 matrix that fits in
SBUF (tiny matmul: S*H*R = 34M MACs). The problem becomes a scalar
gather of N entries from G. Since idx pairs are batch-independent, both
batches' tables are interleaved in SBUF ([128, 8192, 2] f32, partition
p = s%128, e = (s//128)*1024 + h) and a single d=2 ap_gather index
fetches BOTH batches' output values: 2 outputs per index.

ap_gather cost is ~27ns per index per 16-partition group (measured),
independent of d/num_elems, so minimizing index count is everything:
131072 idx/group (baseline) -> 18432 here (7.1x).

Sharding: core c handles n in [c*N/8, (c+1)*N/8) for both batches.
Host buckets each core's 131072 outputs by partition p (pad each bucket
to L=1152), streams group g's 16 buckets lane-by-lane; round r gathers
lane r's L indices for all groups and a static block-indicator matmul
(lhsT ind[:, r]) extracts lane r of each group -> psum [8, 2L] -> out.
Host inverse-permutes the bucketed outputs (pure data movement).
"""

import numpy as np
import ml_dtypes
from contextlib import ExitStack

import concourse.bass as bass
import concourse.mybir as mybir
import concourse.tile as tile
from concourse import bacc

B, S, H, R = 2, 1024, 1024, 32
N = S * H
NCORES = 8
J0 = N // NCORES          # 131072 n-indices per core (serves both batches)
L = 1152                  # padded per-partition bucket length (mean 1024)
RNDS = 16                 # one gather round per lane
NE = 8 * 1024             # d=2 table blocks per partition
OUTW = RNDS * L * 2       # 36864 output cols per core: [8, OUTW]
IDXC = RNDS * L // 16     # idx columns per partition

F32 = mybir.dt.float32
BF16 = mybir.dt.bfloat16
I16 = mybir.dt.int16


def _build(reps: int = 1):
    nc = bacc.Bacc()
    x0 = nc.declare_dram_parameter("x0", [S, H], BF16, False)
    x1 = nc.declare_dram_parameter("x1", [S, H], BF16, False)
    wseq_t = nc.declare_dram_parameter("wseq_t", [H, R], BF16, False)
    whid_t = nc.declare_dram_parameter("whid_t", [S, R], BF16, False)
    idx = nc.declare_dram_parameter("idx", [128, IDXC], I16, False)
    ind_in = nc.declare_dram_parameter("ind", [128, 128], F32, False)
    out = nc.declare_dram_parameter("out", [8, OUTW], F32, True)
    xs = (x0, x1)

    with tile.TileContext(nc) as tc, ExitStack() as ctx:
        base = ctx.enter_context(tc.tile_pool(name="base", bufs=1))
        fps = ctx.enter_context(tc.tile_pool(name="fps", bufs=1, space="PSUM"))
        gps = ctx.enter_context(tc.tile_pool(name="gps", bufs=2, space="PSUM"))
        rps = ctx.enter_context(tc.tile_pool(name="rps", bufs=1, space="PSUM"))
        tabp = ctx.enter_context(tc.tile_pool(name="tabp", bufs=1))
        facp = ctx.enter_context(tc.tile_pool(name="facp", bufs=1))
        gap = ctx.enter_context(tc.tile_pool(name="gap", bufs=2))
        otp = ctx.enter_context(tc.tile_pool(name="otp", bufs=2))

        # --- static loads -----------------------------------------------
        ws_sb = base.tile([128, 8, R], BF16)     # W_seq^T rows, h-major
        wh_sb = base.tile([128, 8, R], BF16)     # W_hid^T rows, s-major
        isb = base.tile([128, IDXC], I16)
        ind_sb = base.tile([128, 128], F32)      # ind[p, 8*l+g]=1 iff p==16g+l
        x_sb = base.tile([128, 2, 8, H], BF16)   # [p, b, k, h]; s = p + 128k
        xt_sb = base.tile([128, 2, 8, S], BF16)  # [p, b, k, s]; h = p + 128k

        nc.sync.dma_start(
            out=ws_sb[:],
            in_=bass.AP(tensor=wseq_t[:].tensor, offset=0,
                        ap=[[R, 128], [128 * R, 8], [1, R]]),
        )
        nc.sync.dma_start(
            out=wh_sb[:],
            in_=bass.AP(tensor=whid_t[:].tensor, offset=0,
                        ap=[[R, 128], [128 * R, 8], [1, R]]),
        )
        nc.sync.dma_start(out=isb[:], in_=idx[:])
        nc.sync.dma_start(out=ind_sb[:], in_=ind_in[:])
        for b in range(B):
            nc.sync.dma_start(
                out=x_sb[:, b, :, :],
                in_=bass.AP(tensor=xs[b][:].tensor, offset=0,
                            ap=[[H, 128], [128 * H, 8], [1, H]]),
            )
            for k in range(8):
                nc.sync.dma_start_transpose(
                    out=xt_sb[:, b, k, :], in_=xs[b][:, 128 * k:128 * (k + 1)]
                )

        for _ in range(reps):
            _body(nc, fps, gps, rps, tabp, facp, gap, otp,
                  ws_sb, wh_sb, isb, ind_sb, x_sb, xt_sb, out)
    nc.compile()
    return nc


def _body(nc, fps, gps, rps, tabp, facp, gap, otp,
          ws_sb, wh_sb, isb, ind_sb, x_sb, xt_sb, out):
    tab = tabp.tile([128, 2 * NE], F32, tag="tab")   # tab[p, 2e+b]
    a_bf = facp.tile([32, 2, S], BF16, tag="a_bf")   # A_b^T[r, s]
    b_bf = facp.tile([32, 2, H], BF16, tag="b_bf")   # Bm_b^T[r, h]

    # --- factor matmuls: F^T [32, 1024] per batch -----------------------
    for b in range(B):
        for (dst, lhs_w, rhs_x) in ((a_bf, ws_sb, xt_sb), (b_bf, wh_sb, x_sb)):
            for nh in range(2):
                pt = fps.tile([R, 512], F32, tag="pt")
                for k in range(8):
                    nc.tensor.matmul(
                        out=pt[:],
                        lhsT=lhs_w[:, k, :],
                        rhs=rhs_x[:, b, k, nh * 512:(nh + 1) * 512],
                        start=(k == 0), stop=(k == 7),
                    )
                nc.vector.tensor_copy(
                    out=dst[:, b, nh * 512:(nh + 1) * 512], in_=pt[:])

    # --- G_b = A_b @ Bm_b^T, written interleaved into tab ---------------
    # block k covers s in [128k, 128k+128): out partition i = s - 128k,
    # table col e = 1024k + h, written at tab[:, 2e + b] (stride 2).
    eng = 0
    for b in range(B):
        for k in range(8):
            for nh in range(2):
                gp = gps.tile([128, 512], F32, tag="gp")
                nc.tensor.matmul(
                    out=gp[:],
                    lhsT=a_bf[:, b, 128 * k:128 * (k + 1)],
                    rhs=b_bf[:, b, 512 * nh:512 * (nh + 1)],
                    start=True, stop=True,
                )
                dst = bass.AP(
                    tensor=tab[:].tensor,
                    offset=tab[:].offset + 2 * (1024 * k + 512 * nh) + b,
                    ap=[list(tab[:].ap[0]), [2, 512]],
                )
                if eng % 2 == 0:
                    nc.vector.tensor_copy(out=dst, in_=gp[:])
                else:
                    nc.scalar.copy(out=dst, in_=gp[:])
                eng += 1

    # --- gather + extract ----------------------------------------------
    tab_flat = bass.AP(tensor=tab[:].tensor, offset=tab[:].offset,
                       ap=[list(tab[:].ap[0]), [1, 2 * NE], [1, 1]])
    ot = None
    for r in range(RNDS):
        ga = gap.tile([128, 2 * L], F32, tag="ga")
        ga_ap = bass.AP(tensor=ga[:].tensor, offset=ga[:].offset,
                        ap=[list(ga[:].ap[0]), [1, 2 * L], [1, 1]])
        nc.gpsimd.ap_gather(
            out_ap=ga_ap, in_ap=tab_flat,
            idxs_ap=isb[:, r * (L // 16):(r + 1) * (L // 16)],
            channels=128, num_elems=NE, d=2, num_idxs=L,
        )
        rp = rps.tile([8, 2 * L], F32, tag="rp")
        for t in range(0, 2 * L, 512):
            w = min(512, 2 * L - t)
            nc.tensor.matmul(
                out=rp[:, t:t + w],
                lhsT=ind_sb[:, 8 * r:8 * (r + 1)],
                rhs=ga[:, t:t + w],
                start=True, stop=True,
            )
        if r % 2 == 0:
            ot = otp.tile([8, 4 * L], F32, tag="ot")
        nc.scalar.copy(out=ot[:, (r % 2) * 2 * L:(r % 2 + 1) * 2 * L],
                       in_=rp[:])
        if r % 2 == 1:
            nc.sync.dma_start(
                out=bass.AP(tensor=out[:].tensor, offset=(r - 1) * 2 * L,
                            ap=[[OUTW, 8], [1, 4 * L]]),
                in_=ot[:],
            )


_nc_cache_by_reps = {}


def _get_nc(reps: int = 1):
    nc = _nc_cache_by_reps.get(reps)
    if nc is None:
        nc = _nc_cache_by_reps[reps] = _build(reps)
    return nc


class _Runner:
    """Trace/compile the SPMD executable once; reuse across calls."""

    def __init__(self, nc):
        import jax
        from jax.experimental.shard_map import shard_map
        from jax.sharding import Mesh, PartitionSpec
        import concourse.bass2jax as b2j

        b2j.install_neuronx_cc_hook()
        self.nc = nc
        part_name = (nc.partition_id_tensor.name
                     if nc.partition_id_tensor else None)
        in_names, out_names, out_avals = [], [], []
        zero_outs = []
        for alloc in nc.m.functions[0].allocations:
            if not isinstance(alloc, mybir.MemoryLocationSet):
                continue
            name = alloc.memorylocations[0].name
            if alloc.kind == "ExternalInput":
                if name != part_name:
                    in_names.append(name)
            elif alloc.kind == "ExternalOutput":
                out_names.append(name)
                shape = tuple(alloc.tensor_shape)
                dtype = mybir.dt.np(alloc.dtype)
                out_avals.append(jax.core.ShapedArray(shape, dtype))
                zero_outs.append(np.zeros(shape, dtype))
        self.in_names = list(in_names)
        self.out_names = out_names
        self.zero_outs = zero_outs
        n_params = len(in_names)
        n_outs = len(out_names)
        all_in_names = in_names + out_names
        if part_name is not None:
            all_in_names = all_in_names + [part_name]
        donate = tuple(range(n_params, n_params + n_outs))

        def _body_fn(*args):
            operands = list(args)
            if part_name is not None:
                operands.append(b2j.partition_id_tensor())
            outs = b2j._bass_exec_p.bind(
                *operands,
                out_avals=tuple(out_avals),
                in_names=tuple(all_in_names),
                out_names=tuple(out_names),
                lowering_input_output_aliases=(),
                sim_require_finite=True,
                sim_require_nnan=True,
                nc=nc,
            )
            return tuple(outs)

        devices = jax.devices()[:NCORES]
        mesh = Mesh(np.asarray(devices), ("core",))
        self.fn = jax.jit(
            shard_map(
                _body_fn, mesh=mesh,
                in_specs=(PartitionSpec("core"),) * (n_params + n_outs),
                out_specs=(PartitionSpec("core"),) * n_outs,
                check_rep=False,
            ),
            donate_argnums=donate,
            keep_unused=True,
        )

    def __call__(self, in_maps):
        concat_in = [
            np.concatenate([np.asarray(m[name]) for m in in_maps], axis=0)
            for name in self.in_names
        ]
        concat_zeros = [
            np.zeros((NCORES * z.shape[0], *z.shape[1:]), z.dtype)
            for z in self.zero_outs
        ]
        out_arrs = self.fn(*concat_in, *concat_zeros)
        return [
            {
                name: np.asarray(out_arrs[i]).reshape(NCORES, -1)[c]
                for i, name in enumerate(self.out_names)
            }
            for c in range(NCORES)
        ]


_runner_cache = {}


def _get_runner(reps: int = 1):
    r = _runner_cache.get(reps)
    if r is None:
        r = _runner_cache[reps] = _Runner(_get_nc(reps))
    return r


def _make_ind() -> np.ndarray:
    ind = np.zeros((128, 128), np.float32)
    for g in range(8):
        for l in range(16):
            ind[16 * g + l, 8 * l + g] = 1.0
    return ind


def _prep_core(s: np.ndarray, h: np.ndarray):
    """Bucket one core's J0 (s, h) pairs by partition p = s % 128.

    Returns (idx_dev [128, IDXC] int16, meta for unpacking).
    """
    p = (s & 127).astype(np.int64)
    e = ((s >> 7) << 10 | h).astype(np.int64)    # (s//128)*1024 + h < 8192
    order = np.argsort(p, kind="stable")
    p_s = p[order]
    cnt = np.bincount(p, minlength=128)
    starts = np.concatenate(([0], np.cumsum(cnt)[:-1]))
    ofs = np.arange(J0) - starts[p_s]
    keep = ofs < L
    arr = np.zeros((128, L), np.int16)
    arr[p_s[keep], ofs[keep]] = e[order][keep].astype(np.int16)
    idx_dev = np.empty((128, IDXC), np.int16)
    for g in range(8):
        stream = arr[16 * g:16 * (g + 1)].reshape(-1)    # lane-major
        idx_dev[16 * g:16 * (g + 1), :] = stream.reshape(-1, 16).T
    return np.ascontiguousarray(idx_dev), (order, p_s, ofs, keep)


def prepare_in_maps(hidden_states, W_seq, W_hid, all_indices):
    hidden_states = np.asarray(hidden_states)
    all_indices = np.asarray(all_indices)
    x_bf = [np.ascontiguousarray(hidden_states[b].astype(ml_dtypes.bfloat16))
            for b in range(B)]
    ws_t = np.ascontiguousarray(np.asarray(W_seq).T.astype(ml_dtypes.bfloat16))
    wh_t = np.ascontiguousarray(np.asarray(W_hid).T.astype(ml_dtypes.bfloat16))
    ind = _make_ind()
    in_maps, metas = [], []
    for c in range(NCORES):
        sl = slice(c * J0, (c + 1) * J0)
        s = all_indices[sl, 0].astype(np.int64)
        h = all_indices[sl, 1].astype(np.int64)
        idx_dev, meta = _prep_core(s, h)
        metas.append(meta)
        in_maps.append({
            "x0": x_bf[0], "x1": x_bf[1],
            "wseq_t": ws_t, "whid_t": wh_t,
            "idx": idx_dev, "ind": ind,
        })
    return in_maps, metas


def _assemble(results, metas, hidden_states, all_indices):
    out_full = np.empty((B, N), dtype=np.float32)
    spill = []
    for c in range(NCORES):
        order, p_s, ofs, keep = metas[c]
        resh = np.asarray(results[c]["out"]).reshape(8, 16, L, 2)
        ps, of = p_s[keep], ofs[keep]
        vals = resh[ps >> 4, ps & 15, of, :]          # [kept, 2]
        n_glob = c * J0 + order[keep]
        out_full[0, n_glob] = vals[:, 0]
        out_full[1, n_glob] = vals[:, 1]
        if not keep.all():
            spill.append(c * J0 + order[~keep])
    if spill:
        # exact host fallback for (astronomically rare) bucket overflow
        ns = np.concatenate(spill)
        s = all_indices[ns, 0].astype(np.int64)
        h = all_indices[ns, 1].astype(np.int64)
        for b in range(B):
            A = hidden_states[b].astype(np.float32) @ np.asarray(
                _W_CACHE["W_seq"]).T.astype(np.float32)
            Bm = hidden_states[b].astype(np.float32).T @ np.asarray(
                _W_CACHE["W_hid"]).T.astype(np.float32)
            out_full[b, ns] = np.sum(A[s] * Bm[h], axis=-1)
    return out_full.reshape(B, S, H)


_W_CACHE = {}


def kernel(hidden_states, W_seq, W_hid, all_indices):
    hidden_states = np.asarray(hidden_states)
    W_seq = np.asarray(W_seq)
    W_hid = np.asarray(W_hid)
    all_indices = np.asarray(all_indices)
    _W_CACHE["W_seq"] = W_seq
    _W_CACHE["W_hid"] = W_hid

    runner = _get_runner()
    in_maps, metas = prepare_in_maps(hidden_states, W_seq, W_hid, all_indices)
    results = runner(in_maps)
    return _assemble(results, metas, hidden_states, all_indices)


# revision 6
# speedup vs baseline: 43.8811x; 1.0478x over previous
"""Trainium2 Bass kernel for nn_CPCircuitLayer (embedding_lookup).

Math: A_b = X_b @ W_seq^T [S,R]; Bm_b = X_b^T @ W_hid^T [H,R]
      out[b, n] = dot(A_b[idx_s[n]], Bm_b[idx_h[n]]),  out -> [B, S, H]

Key reformulation: out[b, n] = G_b[idx_s[n], idx_h[n]] where
G_b = A_b @ Bm_b^T is a [S, H] = [1024, 1024] f32 matrix that fits in
SBUF (tiny matmul: S*H*R = 34M MACs). The problem becomes a scalar
gather of N entries from G. Since idx pairs are batch-independent, both
batches' tables are interleaved in SBUF ([128, 8192, 2] f32, partition
p = s'%128, e = (s'//128)*1024 + h) and a single d=2 ap_gather index
fetches BOTH batches' output values: 2 outputs per index.

ap_gather costs ~27ns per index per 16-partition group (measured),
independent of d/num_elems, so index count is everything: 131072
idx/group (baseline) -> 16640 here (7.9x).

Load balancing: the host permutes X's rows (and W_hid's columns to
match) per core so the 1024 s-rows pack into 128 partitions with
near-equal gather-bucket sizes -> pad L=1040 (~1.5% waste).

Sharding: core c handles n in [c*N/8, (c+1)*N/8) for both batches.
Host buckets each core's 131072 outputs by partition p, pads each
bucket to L, streams group g's 16 buckets lane-by-lane; round r
gathers lane r's L indices for all groups and a static block-indicator
matmul (lhsT = ind[:, 8r:8r+8]) extracts lane r of each group ->
psum [8, 512]-chunks -> out. Host inverse-permutes the bucketed
outputs (pure data movement).

X^T (needed for the A factor) is produced on the PE via transpose
matmuls for batch 0 while batch 1 is DMA-transposed concurrently.
"""

import numpy as np
import ml_dtypes
from contextlib import ExitStack

import concourse.bass as bass
import concourse.mybir as mybir
import concourse.tile as tile
from concourse import bacc

B, S, H, R = 2, 1024, 1024, 32
N = S * H
NCORES = 8
J0 = N // NCORES          # 131072 n-indices per core (serves both batches)
L = 1056                  # padded bucket length (mean 1024); L/16 must be
                          # EVEN so per-round int16 idx slices stay 4B-aligned
RNDS = 16                 # one gather round per lane
NE = 8 * 1024             # d=2 table blocks per partition
OUTW = RNDS * L * 2       # 33280 output cols per core: [8, OUTW]
IDXC = RNDS * L // 16     # idx columns per partition

F32 = mybir.dt.float32
BF16 = mybir.dt.bfloat16
I16 = mybir.dt.int16


def _build(reps: int = 1):
    nc = bacc.Bacc()
    x0 = nc.declare_dram_parameter("x0", [S, H], BF16, False)
    x1 = nc.declare_dram_parameter("x1", [S, H], BF16, False)
    wseq_t = nc.declare_dram_parameter("wseq_t", [H, R], BF16, False)
    whid_t = nc.declare_dram_parameter("whid_t", [S, R], BF16, False)
    idx = nc.declare_dram_parameter("idx", [128, IDXC], I16, False)
    ind_in = nc.declare_dram_parameter("ind", [128, 128], F32, False)
    ident_in = nc.declare_dram_parameter("ident", [128, 128], BF16, False)
    out = nc.declare_dram_parameter("out", [8, OUTW], F32, True)
    xs = (x0, x1)

    with tile.TileContext(nc) as tc, ExitStack() as ctx:
        base = ctx.enter_context(tc.tile_pool(name="base", bufs=1))
        tps = ctx.enter_context(tc.tile_pool(name="tps", bufs=2, space="PSUM"))
        fps = ctx.enter_context(tc.tile_pool(name="fps", bufs=1, space="PSUM"))
        gps = ctx.enter_context(tc.tile_pool(name="gps", bufs=2, space="PSUM"))
        rps = ctx.enter_context(tc.tile_pool(name="rps", bufs=2, space="PSUM"))
        tabp = ctx.enter_context(tc.tile_pool(name="tabp", bufs=1))
        facp = ctx.enter_context(tc.tile_pool(name="facp", bufs=1))
        gap = ctx.enter_context(tc.tile_pool(name="gap", bufs=2))
        otp = ctx.enter_context(tc.tile_pool(name="otp", bufs=2))

        # --- static loads -----------------------------------------------
        ws_sb = base.tile([128, 8, R], BF16)     # W_seq^T rows, h-major
        wh_sb = base.tile([128, 8, R], BF16)     # W_hid^T rows, s-major
        isb = base.tile([128, IDXC], I16)
        ind_sb = base.tile([128, 128], F32)      # ind[p, 8*l+g]=1 iff p==16g+l
        id_sb = base.tile([128, 128], BF16)      # identity for PE transpose
        x_sb = base.tile([128, 2, 8, H], BF16)   # [p, b, k, h]; s' = p + 128k
        xt_sb = base.tile([128, 2, 8, S], BF16)  # [p, b, c, s]; h = p + 128c

        nc.sync.dma_start(
            out=ws_sb[:],
            in_=bass.AP(tensor=wseq_t[:].tensor, offset=0,
                        ap=[[R, 128], [128 * R, 8], [1, R]]),
        )
        nc.sync.dma_start(
            out=wh_sb[:],
            in_=bass.AP(tensor=whid_t[:].tensor, offset=0,
                        ap=[[R, 128], [128 * R, 8], [1, R]]),
        )
        nc.sync.dma_start(out=isb[:], in_=idx[:])
        nc.sync.dma_start(out=ind_sb[:], in_=ind_in[:])
        nc.sync.dma_start(out=id_sb[:], in_=ident_in[:])
        for b in range(B):
            nc.sync.dma_start(
                out=x_sb[:, b, :, :],
                in_=bass.AP(tensor=xs[b][:].tensor, offset=0,
                            ap=[[H, 128], [128 * H, 8], [1, H]]),
            )
        # X^T via HWDGE transpose DMA (reads DRAM directly); batch 0 on
        # the SP queue, batch 1 on the ACT queue -> both walls overlap.
        for k in range(8):
            nc.sync.dma_start_transpose(
                out=xt_sb[:, 0, k, :], in_=x0[:, 128 * k:128 * (k + 1)]
            )
            nc.scalar.dma_start_transpose(
                out=xt_sb[:, 1, k, :], in_=x1[:, 128 * k:128 * (k + 1)]
            )

        for _ in range(reps):
            _body(nc, tps, fps, gps, rps, tabp, facp, gap, otp,
                  ws_sb, wh_sb, isb, ind_sb, id_sb, x_sb, xt_sb, out)
    nc.compile()
    return nc


def _factors_and_g(nc, fps, gps, b, ws_sb, wh_sb, x_sb, xt_sb,
                   a_bf, b_bf, tab, eng):
    # factor matmuls: F^T [32, 1024]
    for (dst, lhs_w, rhs_x) in ((a_bf, ws_sb, xt_sb), (b_bf, wh_sb, x_sb)):
        for nh in range(2):
            pt = fps.tile([R, 512], F32, tag="pt")
            for k in range(8):
                nc.tensor.matmul(
                    out=pt[:],
                    lhsT=lhs_w[:, k, :],
                    rhs=rhs_x[:, b, k, nh * 512:(nh + 1) * 512],
                    start=(k == 0), stop=(k == 7),
                )
            nc.vector.tensor_copy(
                out=dst[:, b, nh * 512:(nh + 1) * 512], in_=pt[:])

    # G_b = A_b @ Bm_b^T, written interleaved into tab:
    # block k covers s' in [128k, 128k+128): out partition i = s' - 128k,
    # table col e = 1024k + h, written at tab[:, 2e + b] (stride 2).
    for k in range(8):
        for nh in range(2):
            gp = gps.tile([128, 512], F32, tag="gp")
            nc.tensor.matmul(
                out=gp[:],
                lhsT=a_bf[:, b, 128 * k:128 * (k + 1)],
                rhs=b_bf[:, b, 512 * nh:512 * (nh + 1)],
                start=True, stop=True,
            )
            dst = bass.AP(
                tensor=tab[:].tensor,
                offset=tab[:].offset + 2 * (1024 * k + 512 * nh) + b,
                ap=[list(tab[:].ap[0]), [2, 512]],
            )
            if eng % 2 == 0:
                nc.vector.tensor_copy(out=dst, in_=gp[:])
            else:
                nc.scalar.copy(out=dst, in_=gp[:])
            eng += 1
    return eng


def _body(nc, tps, fps, gps, rps, tabp, facp, gap, otp,
          ws_sb, wh_sb, isb, ind_sb, id_sb, x_sb, xt_sb, out):
    tab = tabp.tile([128, 2 * NE], F32, tag="tab")   # tab[p, 2e+b]
    a_bf = facp.tile([32, 2, S], BF16, tag="a_bf")   # A_b^T[r, s']
    b_bf = facp.tile([32, 2, H], BF16, tag="b_bf")   # Bm_b^T[r, h]

    eng = 0
    for b in range(B):
        eng = _factors_and_g(nc, fps, gps, b, ws_sb, wh_sb, x_sb, xt_sb,
                             a_bf, b_bf, tab, eng)

    # --- gather + extract ----------------------------------------------
    tab_flat = bass.AP(tensor=tab[:].tensor, offset=tab[:].offset,
                       ap=[list(tab[:].ap[0]), [1, 2 * NE], [1, 1]])
    ot = None
    for r in range(RNDS):
        ga = gap.tile([128, 2 * L], F32, tag="ga")
        ga_ap = bass.AP(tensor=ga[:].tensor, offset=ga[:].offset,
                        ap=[list(ga[:].ap[0]), [1, 2 * L], [1, 1]])
        nc.gpsimd.ap_gather(
            out_ap=ga_ap, in_ap=tab_flat,
            idxs_ap=isb[:, r * (L // 16):(r + 1) * (L // 16)],
            channels=128, num_elems=NE, d=2, num_idxs=L,
        )
        if r % 2 == 0:
            ot = otp.tile([8, 4 * L], F32, tag="ot")
        for t in range(0, 2 * L, 512):
            w = min(512, 2 * L - t)
            rp = rps.tile([8, 512], F32, tag="rp")
            nc.tensor.matmul(
                out=rp[:, :w],
                lhsT=ind_sb[:, 8 * r:8 * (r + 1)],
                rhs=ga[:, t:t + w],
                start=True, stop=True,
            )
            nc.scalar.copy(out=ot[:, (r % 2) * 2 * L + t:
                               (r % 2) * 2 * L + t + w], in_=rp[:, :w])
        if r % 2 == 1:
            nc.sync.dma_start(
                out=bass.AP(tensor=out[:].tensor, offset=(r - 1) * 2 * L,
                            ap=[[OUTW, 8], [1, 4 * L]]),
                in_=ot[:],
            )


_nc_cache_by_reps = {}


def _get_nc(reps: int = 1):
    nc = _nc_cache_by_reps.get(reps)
    if nc is None:
        nc = _nc_cache_by_reps[reps] = _build(reps)
    return nc


class _Runner:
    """Trace/compile the SPMD executable once; reuse across calls."""

    def __init__(self, nc):
        import jax
        from jax.experimental.shard_map import shard_map
        from jax.sharding import Mesh, PartitionSpec
        import concourse.bass2jax as b2j

        b2j.install_neuronx_cc_hook()
        self.nc = nc
        part_name = (nc.partition_id_tensor.name
                     if nc.partition_id_tensor else None)
        in_names, out_names, out_avals = [], [], []
        zero_outs = []
        for alloc in nc.m.functions[0].allocations:
            if not isinstance(alloc, mybir.MemoryLocationSet):
                continue
            name = alloc.memorylocations[0].name
            if alloc.kind == "ExternalInput":
                if name != part_name:
                    in_names.append(name)
            elif alloc.kind == "ExternalOutput":
                out_names.append(name)
                shape = tuple(alloc.tensor_shape)
                dtype = mybir.dt.np(alloc.dtype)
                out_avals.append(jax.core.ShapedArray(shape, dtype))
                zero_outs.append(np.zeros(shape, dtype))
        self.in_names = list(in_names)
        self.out_names = out_names
        self.zero_outs = zero_outs
        n_params = len(in_names)
        n_outs = len(out_names)
        all_in_names = in_names + out_names
        if part_name is not None:
            all_in_names = all_in_names + [part_name]
        donate = tuple(range(n_params, n_params + n_outs))

        def _body_fn(*args):
            operands = list(args)
            if part_name is not None:
                operands.append(b2j.partition_id_tensor())
            outs = b2j._bass_exec_p.bind(
                *operands,
                out_avals=tuple(out_avals),
                in_names=tuple(all_in_names),
                out_names=tuple(out_names),
                lowering_input_output_aliases=(),
                sim_require_finite=True,
                sim_require_nnan=True,
                nc=nc,
            )
            return tuple(outs)

        devices = jax.devices()[:NCORES]
        mesh = Mesh(np.asarray(devices), ("core",))
        self.fn = jax.jit(
            shard_map(
                _body_fn, mesh=mesh,
                in_specs=(PartitionSpec("core"),) * (n_params + n_outs),
                out_specs=(PartitionSpec("core"),) * n_outs,
                check_rep=False,
            ),
            donate_argnums=donate,
            keep_unused=True,
        )

    def __call__(self, in_maps):
        concat_in = [
            np.concatenate([np.asarray(m[name]) for m in in_maps], axis=0)
            for name in self.in_names
        ]
        concat_zeros = [
            np.zeros((NCORES * z.shape[0], *z.shape[1:]), z.dtype)
            for z in self.zero_outs
        ]
        out_arrs = self.fn(*concat_in, *concat_zeros)
        return [
            {
                name: np.asarray(out_arrs[i]).reshape(NCORES, -1)[c]
                for i, name in enumerate(self.out_names)
            }
            for c in range(NCORES)
        ]


_runner_cache = {}


def _get_runner(reps: int = 1):
    r = _runner_cache.get(reps)
    if r is None:
        r = _runner_cache[reps] = _Runner(_get_nc(reps))
    return r


def _make_ind() -> np.ndarray:
    ind = np.zeros((128, 128), np.float32)
    for g in range(8):
        for l in range(16):
            ind[16 * g + l, 8 * l + g] = 1.0
    return ind


def _balance_rows(s: np.ndarray) -> np.ndarray:
    """Assign the 1024 s-rows to 128 partitions (8 rows each) balancing
    total index counts. Returns sigma: sigma[s'] = original row at
    permuted position s' (partition p = s'%128, slot j = s'//128)."""
    rc = np.bincount(s, minlength=S)
    order = np.argsort(-rc, kind="stable")
    bins = np.zeros(128, np.int64)
    slots = np.zeros(128, np.int64)
    sigma = np.empty(S, np.int64)
    for row in order:
        cand = np.flatnonzero(slots < 8)
        p = cand[np.argmin(bins[cand])]
        sigma[p + 128 * slots[p]] = row
        bins[p] += rc[row]
        slots[p] += 1
    return sigma


def _prep_core(s: np.ndarray, h: np.ndarray):
    """Balance + bucket one core's J0 (s, h) pairs.

    Returns (sigma, idx_dev [128, IDXC] int16, meta for unpacking).
    """
    sigma = _balance_rows(s)
    invpos = np.empty(S, np.int64)
    invpos[sigma] = np.arange(S)
    sp = invpos[s]                       # permuted row position s'
    p = sp & 127
    e = ((sp >> 7) << 10) | h            # (s'//128)*1024 + h < 8192
    order = np.argsort(p, kind="stable")
    p_s = p[order]
    cnt = np.bincount(p, minlength=128)
    starts = np.concatenate(([0], np.cumsum(cnt)[:-1]))
    ofs = np.arange(J0) - starts[p_s]
    keep = ofs < L
    arr = np.zeros((128, L), np.int16)
    arr[p_s[keep], ofs[keep]] = e[order][keep].astype(np.int16)
    idx_dev = np.empty((128, IDXC), np.int16)
    for g in range(8):
        stream = arr[16 * g:16 * (g + 1)].reshape(-1)    # lane-major
        idx_dev[16 * g:16 * (g + 1), :] = stream.reshape(-1, 16).T
    return sigma, np.ascontiguousarray(idx_dev), (order, p_s, ofs, keep)


def prepare_in_maps(hidden_states, W_seq, W_hid, all_indices):
    hidden_states = np.asarray(hidden_states)
    all_indices = np.asarray(all_indices)
    x_bf = [hidden_states[b].astype(ml_dtypes.bfloat16) for b in range(B)]
    ws_t = np.ascontiguousarray(np.asarray(W_seq).T.astype(ml_dtypes.bfloat16))
    wh_t_full = np.asarray(W_hid).T.astype(ml_dtypes.bfloat16)  # [S, R]
    ind = _make_ind()
    ident = np.eye(128, dtype=ml_dtypes.bfloat16)
    in_maps, metas = [], []
    for c in range(NCORES):
        sl = slice(c * J0, (c + 1) * J0)
        s = all_indices[sl, 0].astype(np.int64)
        h = all_indices[sl, 1].astype(np.int64)
        sigma, idx_dev, meta = _prep_core(s, h)
        metas.append(meta)
        in_maps.append({
            "x0": np.ascontiguousarray(x_bf[0][sigma]),
            "x1": np.ascontiguousarray(x_bf[1][sigma]),
            "wseq_t": ws_t,
            "whid_t": np.ascontiguousarray(wh_t_full[sigma]),
            "idx": idx_dev, "ind": ind, "ident": ident,
        })
    return in_maps, metas


def _assemble(results, metas, hidden_states, all_indices):
    out_full = np.empty((B, N), dtype=np.float32)
    spill = []
    for c in range(NCORES):
        order, p_s, ofs, keep = metas[c]
        resh = np.asarray(results[c]["out"]).reshape(8, 16, L, 2)
        ps, of = p_s[keep], ofs[keep]
        vals = resh[ps >> 4, ps & 15, of, :]          # [kept, 2]
        n_glob = c * J0 + order[keep]
        out_full[0, n_glob] = vals[:, 0]
        out_full[1, n_glob] = vals[:, 1]
        if not keep.all():
            spill.append(c * J0 + order[~keep])
    if spill:
        # exact host fallback for (astronomically rare) bucket overflow
        ns = np.concatenate(spill)
        s = all_indices[ns, 0].astype(np.int64)
        h = all_indices[ns, 1].astype(np.int64)
        for b in range(B):
            A = hidden_states[b].astype(np.float32) @ np.asarray(
                _W_CACHE["W_seq"]).T.astype(np.float32)
            Bm = hidden_states[b].astype(np.float32).T @ np.asarray(
                _W_CACHE["W_hid"]).T.astype(np.float32)
            out_full[b, ns] = np.sum(A[s] * Bm[h], axis=-1)
    return out_full.reshape(B, S, H)


_W_CACHE = {}


def kernel(hidden_states, W_seq, W_hid, all_indices):
    hidden_states = np.asarray(hidden_states)
    W_seq = np.asarray(W_seq)
    W_hid = np.asarray(W_hid)
    all_indices = np.asarray(all_indices)
    _W_CACHE["W_seq"] = W_seq
    _W_CACHE["W_hid"] = W_hid

    runner = _get_runner()
    in_maps, metas = prepare_in_maps(hidden_states, W_seq, W_hid, all_indices)
    results = runner(in_maps)
    return _assemble(results, metas, hidden_states, all_indices)


# revision 11
# speedup vs baseline: 49.4134x; 1.1261x over previous
"""Trainium2 Bass kernel for nn_CPCircuitLayer (embedding_lookup).

Math: A_b = X_b @ W_seq^T [S,R]; Bm_b = X_b^T @ W_hid^T [H,R]
      out[b, n] = dot(A_b[idx_s[n]], Bm_b[idx_h[n]]),  out -> [B, S, H]

Key reformulation: out[b, n] = G_b[idx_s[n], idx_h[n]] where
G_b = A_b @ Bm_b^T is a [S, H] = [1024, 1024] f32 matrix that fits in
SBUF (tiny matmul: S*H*R = 34M MACs). The problem becomes a scalar
gather of N entries from G. Since idx pairs are batch-independent, both
batches' tables are interleaved in SBUF ([128, 8192, 2] f32, partition
p = s'%128, e = (s'//128)*1024 + h) and a single d=2 ap_gather index
fetches BOTH batches' output values: 2 outputs per index.

ap_gather costs ~27ns per index per 16-partition group (measured),
independent of d/num_elems, so index count is everything: 131072
idx/group (baseline) -> 16640 here (7.9x).

Load balancing: the host permutes X's rows (and W_hid's columns to
match) per core so the 1024 s-rows pack into 128 partitions with
near-equal gather-bucket sizes -> pad L=1040 (~1.5% waste).

Sharding: core c handles n in [c*N/8, (c+1)*N/8) for both batches.
Host buckets each core's 131072 outputs by partition p, pads each
bucket to L, streams group g's 16 buckets lane-by-lane; round r
gathers lane r's L indices for all groups and a static block-indicator
matmul (lhsT = ind[:, 8r:8r+8]) extracts lane r of each group ->
psum [8, 512]-chunks -> out. Host inverse-permutes the bucketed
outputs (pure data movement).

X^T (needed for the A factor) is produced on the PE via transpose
matmuls for batch 0 while batch 1 is DMA-transposed concurrently.
"""

import numpy as np
import ml_dtypes
from contextlib import ExitStack

import concourse.bass as bass
import concourse.mybir as mybir
import concourse.tile as tile
from concourse import bacc

B, S, H, R = 2, 1024, 1024, 32
N = S * H
NCORES = 8
J0 = N // NCORES          # 131072 n-indices per core (serves both batches)
L = 1040                  # padded per-partition bucket length (mean 1024)
RNDS = 16                 # one gather round per lane
NE = 8 * 1024             # d=2 table blocks per partition
OUTW = RNDS * L * 2       # 33280 output cols per core: [8, OUTW]
IDXB = L // 16 + 1        # 66: idx cols per round band, padded so every
                          # band starts 4B-aligned (int16 stream)

F32 = mybir.dt.float32
BF16 = mybir.dt.bfloat16
I16 = mybir.dt.int16


def _build(reps: int = 1):
    nc = bacc.Bacc()
    x0 = nc.declare_dram_parameter("x0", [S, H], BF16, False)
    x1 = nc.declare_dram_parameter("x1", [S, H], BF16, False)
    wseq_t = nc.declare_dram_parameter("wseq_t", [H, R], BF16, False)
    whid_t = nc.declare_dram_parameter("whid_t", [S, R], BF16, False)
    idx = nc.declare_dram_parameter("idx", [128, RNDS * IDXB], I16, False)
    ind_in = nc.declare_dram_parameter("ind", [128, 128], BF16, False)
    ident_in = nc.declare_dram_parameter("ident", [128, 128], BF16, False)
    out = nc.declare_dram_parameter("out", [8, OUTW], F32, True)
    xs = (x0, x1)

    with tile.TileContext(nc) as tc, ExitStack() as ctx:
        base = ctx.enter_context(tc.tile_pool(name="base", bufs=1))
        tps = ctx.enter_context(tc.tile_pool(name="tps", bufs=2, space="PSUM"))
        fps = ctx.enter_context(tc.tile_pool(name="fps", bufs=1, space="PSUM"))
        gps = ctx.enter_context(tc.tile_pool(name="gps", bufs=2, space="PSUM"))
        rps = ctx.enter_context(tc.tile_pool(name="rps", bufs=2, space="PSUM"))
        tabp = ctx.enter_context(tc.tile_pool(name="tabp", bufs=1))
        facp = ctx.enter_context(tc.tile_pool(name="facp", bufs=1))
        gap = ctx.enter_context(tc.tile_pool(name="gap", bufs=2))
        otp = ctx.enter_context(tc.tile_pool(name="otp", bufs=2))

        # --- static loads -----------------------------------------------
        ws_sb = base.tile([128, 8, R], BF16)     # W_seq^T rows, h-major
        wh_sb = base.tile([128, 8, R], BF16)     # W_hid^T rows, s-major
        isb = base.tile([128, RNDS, IDXB], I16)
        ind_sb = base.tile([128, 128], BF16)     # ind[p, 8*l+g]=1 iff p==16g+l
        id_sb = base.tile([128, 128], BF16)      # identity for PE transpose
        x_sb = base.tile([128, 2, 8, H], BF16)   # [p, b, k, h]; s' = p + 128k
        xt_sb = base.tile([128, 2, 8, S], BF16)  # [p, b, c, s]; h = p + 128c

        nc.sync.dma_start(
            out=ws_sb[:],
            in_=bass.AP(tensor=wseq_t[:].tensor, offset=0,
                        ap=[[R, 128], [128 * R, 8], [1, R]]),
        )
        nc.sync.dma_start(
            out=wh_sb[:],
            in_=bass.AP(tensor=whid_t[:].tensor, offset=0,
                        ap=[[R, 128], [128 * R, 8], [1, R]]),
        )
        nc.sync.dma_start(
            out=isb[:],
            in_=bass.AP(tensor=idx[:].tensor, offset=0,
                        ap=[[RNDS * IDXB, 128], [IDXB, RNDS], [1, IDXB]]),
        )
        nc.sync.dma_start(out=ind_sb[:], in_=ind_in[:])
        nc.sync.dma_start(out=id_sb[:], in_=ident_in[:])
        # chunked x loads so the PE pipeline can start on chunk 0
        for b in range(B):
            for k in range(8):
                nc.sync.dma_start(
                    out=x_sb[:, b, k, :],
                    in_=bass.AP(tensor=xs[b][:].tensor, offset=128 * k * H,
                                ap=[[H, 128], [1, H]]),
                )

        for _ in range(reps):
            _body(nc, tps, fps, gps, rps, tabp, facp, gap, otp,
                  ws_sb, wh_sb, isb, ind_sb, id_sb, x_sb, xt_sb, out)
    nc.compile()
    return nc


def _body(nc, tps, fps, gps, rps, tabp, facp, gap, otp,
          ws_sb, wh_sb, isb, ind_sb, id_sb, x_sb, xt_sb, out):
    tab = tabp.tile([128, 2 * NE], BF16, tag="tab")  # tab[p, 2e+b]
    a_bf = facp.tile([32, 2, S], BF16, tag="a_bf")   # A_b^T[r, s']
    b_bf = facp.tile([32, 2, H], BF16, tag="b_bf")   # Bm_b^T[r, h]

    # --- phase 1: factor B + X^T transposes, chunk-pipelined ------------
    # Both consume x chunk (b, k) only, so they start as soon as the
    # chunk's DMA lands.
    for b in range(B):
        ptb = fps.tile([R, 1024], F32, tag="pt")
        for k in range(8):
            for nh in range(2):
                nc.tensor.matmul(
                    out=ptb[:, nh * 512:(nh + 1) * 512],
                    lhsT=wh_sb[:, k, :],
                    rhs=x_sb[:, b, k, nh * 512:(nh + 1) * 512],
                    start=(k == 0), stop=(k == 7),
                )
            tp = tps.tile([128, 8, 128], BF16, tag="tp")
            for c in range(8):
                nc.tensor.transpose(
                    out=tp[:, c, :],
                    in_=x_sb[:, b, k, 128 * c:128 * (c + 1)],
                    identity=id_sb[:],
                )
            nc.vector.tensor_copy(
                out=xt_sb[:, b, :, 128 * k:128 * (k + 1)], in_=tp[:])
        nc.scalar.copy(out=b_bf[:, b, :], in_=ptb[:])

    # --- phase 2: factor A (needs X^T) ----------------------------------
    for b in range(B):
        pta = fps.tile([R, 1024], F32, tag="pt")
        for k in range(8):
            for nh in range(2):
                nc.tensor.matmul(
                    out=pta[:, nh * 512:(nh + 1) * 512],
                    lhsT=ws_sb[:, k, :],
                    rhs=xt_sb[:, b, k, nh * 512:(nh + 1) * 512],
                    start=(k == 0), stop=(k == 7),
                )
        nc.vector.tensor_copy(out=a_bf[:, b, :], in_=pta[:])

    # --- phase 3: G_b = A_b @ Bm_b^T, interleaved into tab --------------
    # block k covers s' in [128k, 128k+128): out partition i = s' - 128k,
    # table col e = 1024k + h, written at tab[:, 2e + b] (stride 2).
    eng = 0
    for b in range(B):
        for k in range(8):
            for nh in range(2):
                gp = gps.tile([128, 512], F32, tag="gp")
                nc.tensor.matmul(
                    out=gp[:],
                    lhsT=a_bf[:, b, 128 * k:128 * (k + 1)],
                    rhs=b_bf[:, b, 512 * nh:512 * (nh + 1)],
                    start=True, stop=True,
                )
                dst = bass.AP(
                    tensor=tab[:].tensor,
                    offset=tab[:].offset + 2 * (1024 * k + 512 * nh) + b,
                    ap=[list(tab[:].ap[0]), [2, 512]],
                )
                if eng % 2 == 0:
                    nc.vector.tensor_copy(out=dst, in_=gp[:])
                else:
                    nc.scalar.copy(out=dst, in_=gp[:])
                eng += 1

    # --- gather + extract ----------------------------------------------
    tab_flat = bass.AP(tensor=tab[:].tensor, offset=tab[:].offset,
                       ap=[list(tab[:].ap[0]), [1, 2 * NE], [1, 1]])
    ot = None
    for r in range(RNDS):
        ga = gap.tile([128, 2 * L], BF16, tag="ga")
        ga_ap = bass.AP(tensor=ga[:].tensor, offset=ga[:].offset,
                        ap=[list(ga[:].ap[0]), [1, 2 * L], [1, 1]])
        nc.gpsimd.ap_gather(
            out_ap=ga_ap, in_ap=tab_flat,
            idxs_ap=isb[:, r, :L // 16],
            channels=128, num_elems=NE, d=2, num_idxs=L,
        )
        if r % 2 == 0:
            ot = otp.tile([8, 4 * L], F32, tag="ot")
        for t in range(0, 2 * L, 512):
            w = min(512, 2 * L - t)
            rp = rps.tile([8, 512], F32, tag="rp")
            nc.tensor.matmul(
                out=rp[:, :w],
                lhsT=ind_sb[:, 8 * r:8 * (r + 1)],
                rhs=ga[:, t:t + w],
                start=True, stop=True,
            )
            nc.scalar.copy(out=ot[:, (r % 2) * 2 * L + t:
                               (r % 2) * 2 * L + t + w], in_=rp[:, :w])
        if r % 2 == 1:
            nc.sync.dma_start(
                out=bass.AP(tensor=out[:].tensor, offset=(r - 1) * 2 * L,
                            ap=[[OUTW, 8], [1, 4 * L]]),
                in_=ot[:],
            )


_nc_cache_by_reps = {}


def _get_nc(reps: int = 1):
    nc = _nc_cache_by_reps.get(reps)
    if nc is None:
        nc = _nc_cache_by_reps[reps] = _build(reps)
    return nc


class _Runner:
    """Trace/compile the SPMD executable once; reuse across calls."""

    def __init__(self, nc):
        import jax
        from jax.experimental.shard_map import shard_map
        from jax.sharding import Mesh, PartitionSpec
        import concourse.bass2jax as b2j

        b2j.install_neuronx_cc_hook()
        self.nc = nc
        part_name = (nc.partition_id_tensor.name
                     if nc.partition_id_tensor else None)
        in_names, out_names, out_avals = [], [], []
        zero_outs = []
        for alloc in nc.m.functions[0].allocations:
            if not isinstance(alloc, mybir.MemoryLocationSet):
                continue
            name = alloc.memorylocations[0].name
            if alloc.kind == "ExternalInput":
                if name != part_name:
                    in_names.append(name)
            elif alloc.kind == "ExternalOutput":
                out_names.append(name)
                shape = tuple(alloc.tensor_shape)
                dtype = mybir.dt.np(alloc.dtype)
                out_avals.append(jax.core.ShapedArray(shape, dtype))
                zero_outs.append(np.zeros(shape, dtype))
        self.in_names = list(in_names)
        self.out_names = out_names
        self.zero_outs = zero_outs
        n_params = len(in_names)
        n_outs = len(out_names)
        all_in_names = in_names + out_names
        if part_name is not None:
            all_in_names = all_in_names + [part_name]
        donate = tuple(range(n_params, n_params + n_outs))

        def _body_fn(*args):
            operands = list(args)
            if part_name is not None:
                operands.append(b2j.partition_id_tensor())
            outs = b2j._bass_exec_p.bind(
                *operands,
                out_avals=tuple(out_avals),
                in_names=tuple(all_in_names),
                out_names=tuple(out_names),
                lowering_input_output_aliases=(),
                sim_require_finite=True,
                sim_require_nnan=True,
                nc=nc,
            )
            return tuple(outs)

        devices = jax.devices()[:NCORES]
        mesh = Mesh(np.asarray(devices), ("core",))
        self.fn = jax.jit(
            shard_map(
                _body_fn, mesh=mesh,
                in_specs=(PartitionSpec("core"),) * (n_params + n_outs),
                out_specs=(PartitionSpec("core"),) * n_outs,
                check_rep=False,
            ),
            donate_argnums=donate,
            keep_unused=True,
        )

    def __call__(self, in_maps):
        concat_in = [
            np.concatenate([np.asarray(m[name]) for m in in_maps], axis=0)
            for name in self.in_names
        ]
        concat_zeros = [
            np.zeros((NCORES * z.shape[0], *z.shape[1:]), z.dtype)
            for z in self.zero_outs
        ]
        out_arrs = self.fn(*concat_in, *concat_zeros)
        return [
            {
                name: np.asarray(out_arrs[i]).reshape(NCORES, -1)[c]
                for i, name in enumerate(self.out_names)
            }
            for c in range(NCORES)
        ]


_runner_cache = {}


def _get_runner(reps: int = 1):
    r = _runner_cache.get(reps)
    if r is None:
        r = _runner_cache[reps] = _Runner(_get_nc(reps))
    return r


def _make_ind() -> np.ndarray:
    ind = np.zeros((128, 128), ml_dtypes.bfloat16)
    for g in range(8):
        for l in range(16):
            ind[16 * g + l, 8 * l + g] = 1.0
    return ind


def _balance_rows(s: np.ndarray) -> np.ndarray:
    """Assign the 1024 s-rows to 128 partitions (8 rows each) balancing
    total index counts. Returns sigma: sigma[s'] = original row at
    permuted position s' (partition p = s'%128, slot j = s'//128)."""
    rc = np.bincount(s, minlength=S)
    order = np.argsort(-rc, kind="stable")
    bins = np.zeros(128, np.int64)
    slots = np.zeros(128, np.int64)
    sigma = np.empty(S, np.int64)
    for row in order:
        cand = np.flatnonzero(slots < 8)
        p = cand[np.argmin(bins[cand])]
        sigma[p + 128 * slots[p]] = row
        bins[p] += rc[row]
        slots[p] += 1
    return sigma


def _prep_core(s: np.ndarray, h: np.ndarray):
    """Balance + bucket one core's J0 (s, h) pairs.

    Returns (sigma, idx_dev [128, RNDS*IDXB] int16, meta for unpacking).
    """
    sigma = _balance_rows(s)
    invpos = np.empty(S, np.int64)
    invpos[sigma] = np.arange(S)
    sp = invpos[s]                       # permuted row position s'
    p = sp & 127
    e = ((sp >> 7) << 10) | h            # (s'//128)*1024 + h < 8192
    order = np.argsort(p, kind="stable")
    p_s = p[order]
    cnt = np.bincount(p, minlength=128)
    starts = np.concatenate(([0], np.cumsum(cnt)[:-1]))
    ofs = np.arange(J0) - starts[p_s]
    keep = ofs < L
    arr = np.zeros((128, L), np.int16)
    arr[p_s[keep], ofs[keep]] = e[order][keep].astype(np.int16)
    idx_dev = np.zeros((128, RNDS, IDXB), np.int16)
    for g in range(8):
        for r in range(RNDS):
            lane = arr[16 * g + r]                       # round r = lane r
            idx_dev[16 * g:16 * (g + 1), r, :L // 16] = \
                lane.reshape(L // 16, 16).T
    idx_dev = idx_dev.reshape(128, RNDS * IDXB)
    return sigma, np.ascontiguousarray(idx_dev), (order, p_s, ofs, keep)


def prepare_in_maps(hidden_states, W_seq, W_hid, all_indices):
    hidden_states = np.asarray(hidden_states)
    all_indices = np.asarray(all_indices)
    x_bf = [hidden_states[b].astype(ml_dtypes.bfloat16) for b in range(B)]
    ws_t = np.ascontiguousarray(np.asarray(W_seq).T.astype(ml_dtypes.bfloat16))
    wh_t_full = np.asarray(W_hid).T.astype(ml_dtypes.bfloat16)  # [S, R]
    ind = _make_ind()
    ident = np.eye(128, dtype=ml_dtypes.bfloat16)
    in_maps, metas = [], []
    for c in range(NCORES):
        sl = slice(c * J0, (c + 1) * J0)
        s = all_indices[sl, 0].astype(np.int64)
        h = all_indices[sl, 1].astype(np.int64)
        sigma, idx_dev, meta = _prep_core(s, h)
        metas.append(meta)
        in_maps.append({
            "x0": np.ascontiguousarray(x_bf[0][sigma]),
            "x1": np.ascontiguousarray(x_bf[1][sigma]),
            "wseq_t": ws_t,
            "whid_t": np.ascontiguousarray(wh_t_full[sigma]),
            "idx": idx_dev, "ind": ind, "ident": ident,
        })
    return in_maps, metas


def _assemble(results, metas, hidden_states, all_indices):
    out_full = np.empty((B, N), dtype=np.float32)
    spill = []
    for c in range(NCORES):
        order, p_s, ofs, keep = metas[c]
        resh = np.asarray(results[c]["out"]).reshape(8, 16, L, 2)
        ps, of = p_s[keep], ofs[keep]
        vals = resh[ps >> 4, ps & 15, of, :]          # [kept, 2]
        n_glob = c * J0 + order[keep]
        out_full[0, n_glob] = vals[:, 0]
        out_full[1, n_glob] = vals[:, 1]
        if not keep.all():
            spill.append(c * J0 + order[~keep])
    if spill:
        # exact host fallback for (astronomically rare) bucket overflow
        ns = np.concatenate(spill)
        s = all_indices[ns, 0].astype(np.int64)
        h = all_indices[ns, 1].astype(np.int64)
        for b in range(B):
            A = hidden_states[b].astype(np.float32) @ np.asarray(
                _W_CACHE["W_seq"]).T.astype(np.float32)
            Bm = hidden_states[b].astype(np.float32).T @ np.asarray(
                _W_CACHE["W_hid"]).T.astype(np.float32)
            out_full[b, ns] = np.sum(A[s] * Bm[h], axis=-1)
    return out_full.reshape(B, S, H)


_W_CACHE = {}


def kernel(hidden_states, W_seq, W_hid, all_indices):
    hidden_states = np.asarray(hidden_states)
    W_seq = np.asarray(W_seq)
    W_hid = np.asarray(W_hid)
    all_indices = np.asarray(all_indices)
    _W_CACHE["W_seq"] = W_seq
    _W_CACHE["W_hid"] = W_hid

    runner = _get_runner()
    in_maps, metas = prepare_in_maps(hidden_states, W_seq, W_hid, all_indices)
    results = runner(in_maps)
    return _assemble(results, metas, hidden_states, all_indices)
